# revision 27
# baseline (speedup 1.0000x reference)
"""FFT-Conv2d (with buggy custom ifft2) — Trainium2 Bass kernel.

Math: the reference's custom ifft2 (missing final conj) reduces, after the
center crop, to
    out[b,o,r,c] = bias[o]                          for r<31 or c>33
    out[b,o,r,c] = bias[o] + g[b,o,96-r,32+c]       for 31<=r<=63, 0<=c<=33
where g = full linear conv of x (64x64) with w (3x3, true convolution),
summed over input channels.  So the device only computes the 33x34 region
g[p=33..65, q=32..65] (+bias); the host assembles the rest (bias broadcast).

Device mapping (per core, 2 batches): contraction K = 32 IC x 3 col-taps
(host-replicated with the column shift baked in) + 1 ones-row carrying the
bias => K=97.  3 accumulating matmuls (one per row-tap u) per chunk of 11
output rows (N=374), 3 chunks per batch.
"""

import os
import numpy as np
from contextlib import ExitStack

import concourse.bacc as bacc
import concourse.tile as tile
from concourse import mybir
from concourse.bass_utils import run_bass_kernel_spmd

N_CORES = 8
B, IC, H, W = 16, 32, 64, 64
OC = 64
BPC = B // N_CORES          # batches per core
KPART = 97                  # 3*32 taps + 1 ones row
PPAD = 128                  # padded partition count for DMA striping
TROWS, RCOLS = 35, 34       # x-patch tile rows (j=0..34 -> x rows 31..65), cols
RROWS = 33                  # g-region rows p = 33..65
CHUNKS = [(0, 14), (14, 14), (28, 5)]   # (row0, nrows): small last chunk
NFREE = 14 * RCOLS          # largest chunk free dim (476)

MM_DT_NAME = os.environ.get("KERNEL_MM_DT", "float16")
N_WARMUP = int(os.environ.get("KERNEL_N_WARMUP", "12"))

_cache = {}


def _mm_dt():
    return {
        "float32": mybir.dt.float32,
        "float32r": mybir.dt.float32r,
        "bfloat16": mybir.dt.bfloat16,
        "float16": mybir.dt.float16,
    }[MM_DT_NAME]


def _np_dt(mdt):
    return mybir.dt.np(mdt)


def _patch_tile_teardown():
    """Drop the second all-engine barrier in TileContext's teardown: the
    sem-range clear runs on Pool after the first barrier; other engines
    need not wait for it (the runtime joins all engine streams at NEFF
    end anyway)."""
    from concourse.vector_clock import ScopedClock

    def _drain_and_barrier(self, tick_clock, wait_clock):
        drain_inst = self.nc.sync.drain()
        wait_clock.add_sem_waits(
            drain_inst.ins, ScopedClock({None: tick_clock.global_clock})
        )
        self.nc.all_engine_barrier()
        popped = self.nc._tile_sem_poison_stack.pop()
        assert popped is self._sem_poison
        self.nc.clear_and_free_semaphores(list(self.sems.allocated().values()))

    tile.TileContext._drain_and_barrier = _drain_and_barrier


_patch_tile_teardown()


def _build(mm_dt):
    # Skip the barrier Bass.__init__ emits after its const-pool memsets —
    # this kernel never reads the const pool from another engine.
    orig_barrier = bacc.Bacc.all_engine_barrier
    bacc.Bacc.all_engine_barrier = lambda self, **kw: None
    try:
        nc = bacc.Bacc(
            "TRN2", target_bir_lowering=False, debug=False, num_devices=N_CORES
        )
    finally:
        bacc.Bacc.all_engine_barrier = orig_barrier
    # batch-0 patch and the weights travel in ONE DMA (wt appended on the
    # free dim) so the weight packets don't compete with xt0's on the
    # SDMA engines
    xtw_d = nc.dram_tensor(
        "xtw", [PPAD, TROWS * RCOLS + 3 * OC], mm_dt, kind="ExternalInput"
    ).ap()
    xt1_d = nc.dram_tensor(
        "xt1", [PPAD, TROWS * RCOLS], mm_dt, kind="ExternalInput"
    ).ap()
    out_d = nc.dram_tensor(
        "out", [BPC, OC, RROWS, RCOLS], mybir.dt.float32, kind="ExternalOutput"
    ).ap()

    with tile.TileContext(nc) as tc, ExitStack() as ctx:
        xt_pool = ctx.enter_context(tc.tile_pool(name="xt", bufs=1))
        ps_pool = ctx.enter_context(tc.tile_pool(name="ps", bufs=6, space="PSUM"))
        ob_pool = ctx.enter_context(tc.tile_pool(name="ob", bufs=6))

        # Warm-up tile memset on ACT (scalar) — earliest free engine; the
        # PE warm-up matmuls flip the HAM clock gate (1.2 -> 2.4 GHz)
        # during the DMA wait so the real matmuls all run warm.
        # raw (non-pool) SBUF buffer: contents are garbage, which is fine
        # for warm-up matmuls, and needs no producing write to schedule
        NWARM = 374
        warm = nc.alloc_sbuf_tensor("warmbuf", [PPAD, NWARM], mm_dt).ap()

        # Inputs: both on gpsimd (SWDGE) — its queue starts emitting
        # earliest after the preamble, and same-queue FIFO drains batch 0
        # (+weights) before batch 1, so batch-0 compute starts at half the
        # transfer time. 128-partition transfers stripe across all 16
        # SDMA engines.
        xtw = xt_pool.tile([PPAD, TROWS * RCOLS + 3 * OC], mm_dt, tag="xtw")
        nc.gpsimd.dma_start(out=xtw[:, :], in_=xtw_d[:, :])
        xt1 = xt_pool.tile([PPAD, TROWS * RCOLS], mm_dt, tag="xt1")
        nc.gpsimd.dma_start(out=xt1[:, :], in_=xt1_d[:, :])
        wt = xtw[:, TROWS * RCOLS : TROWS * RCOLS + 3 * OC]
        xts = [xtw, xt1]

        wps = ps_pool.tile([OC, NWARM], mybir.dt.float32, tag="warmps", bufs=1)
        for _ in range(N_WARMUP):
            nc.tensor.matmul(
                wps[:, :], warm[:, 0:OC], warm[:, :], start=True, stop=True
            )

        for b in range(BPC):
            xt = xts[b]
            for ch, (r0, nr) in enumerate(CHUNKS):
                nf = nr * RCOLS
                ps = ps_pool.tile([OC, NFREE], mybir.dt.float32)
                for u in range(3):
                    # chunk covers p = 33+r0 .. +nr-1; tile row j = p-u-31
                    j0 = 2 + r0 - u
                    kk = KPART if u == 0 else KPART - 1
                    nc.tensor.matmul(
                        ps[:, 0:nf],
                        wt[0:kk, u * OC : (u + 1) * OC],
                        xt[0:kk, j0 * RCOLS : j0 * RCOLS + nf],
                        start=(u == 0),
                        stop=(u == 2),
                    )
                ob = ob_pool.tile([OC, NFREE], mybir.dt.float32)
                # split each psum->sbuf copy across DVE and ACT: halves the
                # copy latency on the critical tail and load-balances both
                # engines during the matmul phase
                half = (nf // 2) // 2 * 2
                nc.vector.tensor_copy(ob[:, 0:half], ps[:, 0:half])
                nc.scalar.copy(ob[:, half:nf], ps[:, half:nf])
                (nc.sync if ch % 2 == 0 else nc.scalar).dma_start(
                    out=out_d[b, :, r0 : r0 + nr, :],
                    in_=ob[:, 0:nf].rearrange("p (r c) -> p r c", c=RCOLS),
                )
    nc.compile()
    return nc


def _get_nc():
    key = MM_DT_NAME
    if key not in _cache:
        _cache[key] = _build(_mm_dt())
    return _cache[key]


LAST_RESULTS = None


def kernel(x, weight, bias):
    global LAST_RESULTS
    x = np.asarray(x, dtype=np.float32)
    weight = np.asarray(weight, dtype=np.float32)
    bias = np.asarray(bias, dtype=np.float32)
    np_dt = _np_dt(_mm_dt())

    # --- host prep: shard + im2col-lite (3 column-shifted replicas) ---
    xpad = np.zeros((B, IC, H + 2, W + 2), np.float32)
    xpad[:, :, :H, :W] = x
    XT = np.zeros((B, PPAD, TROWS, RCOLS), np.float32)
    for v in range(3):
        XT[:, v * 32 : (v + 1) * 32, :, :] = xpad[
            :, :, 31 : 31 + TROWS, 32 - v : 32 - v + RCOLS
        ]
    XT[:, 96] = 1.0
    XT = np.ascontiguousarray(XT.reshape(B, PPAD, TROWS * RCOLS)).astype(np_dt)

    WT = np.zeros((PPAD, 3 * OC), np.float32)
    # WT[v*32+i, u*64+oc] = weight[oc,i,u,v]
    WT[:96, :] = weight.transpose(3, 1, 2, 0).reshape(96, 3 * OC)
    WT[96, 0:OC] = bias
    WT = WT.astype(np_dt)

    nc = _get_nc()
    in_maps = [
        {
            "xtw": np.ascontiguousarray(
                np.concatenate([XT[c * BPC], WT], axis=1)
            ),
            "xt1": XT[c * BPC + 1],
        }
        for c in range(N_CORES)
    ]
    res = run_bass_kernel_spmd(nc, in_maps, list(range(N_CORES)))
    LAST_RESULTS = res

    dev = np.stack([r["out"] for r in res.results])  # [8, BPC, OC, 33, 34]
    dev = dev.reshape(B, OC, RROWS, RCOLS)

    # --- host assembly: bias everywhere, conv region flipped in ---
    full = np.empty((B, OC, H, W), np.float32)
    full[:] = bias[None, :, None, None]
    full[:, :, 31:64, 0:34] = dev[:, :, ::-1, :]
    return full


# revision 29
# speedup vs baseline: 1.0742x; 1.0742x over previous
"""FFT-Conv2d (with buggy custom ifft2) — Trainium2 Bass kernel.

Math: the reference's custom ifft2 (missing final conj) reduces, after the
center crop, to
    out[b,o,r,c] = bias[o]                          for r<31 or c>33
    out[b,o,r,c] = bias[o] + g[b,o,96-r,32+c]       for 31<=r<=63, 0<=c<=33
where g = full linear conv of x (64x64) with w (3x3, true convolution),
summed over input channels.  So the device only computes the 33x34 region
g[p=33..65, q=32..65] (+bias); the host assembles the rest (bias broadcast).

Device mapping (per core, 2 batches): contraction K = 32 IC x 3 col-taps
(host-replicated with the column shift baked in) + 1 ones-row carrying the
bias => K=97.  3 accumulating matmuls (one per row-tap u) per chunk of 11
output rows (N=374), 3 chunks per batch.
"""

import os
import numpy as np
from contextlib import ExitStack

import concourse.bacc as bacc
import concourse.tile as tile
from concourse import mybir
from concourse.bass_utils import run_bass_kernel_spmd

N_CORES = 8
B, IC, H, W = 16, 32, 64, 64
OC = 64
BPC = B // N_CORES          # batches per core
KPART = 97                  # 3*32 taps + 1 ones row
PPAD = 128                  # padded partition count for DMA striping
TROWS, RCOLS = 35, 34       # x-patch tile rows (j=0..34 -> x rows 31..65), cols
RROWS = 33                  # g-region rows p = 33..65
CHUNKS = [(0, 14), (14, 14), (28, 5)]   # (row0, nrows): small last chunk
NFREE = 14 * RCOLS          # largest chunk free dim (476)

MM_DT_NAME = os.environ.get("KERNEL_MM_DT", "float16")
N_WARMUP = int(os.environ.get("KERNEL_N_WARMUP", "12"))

_cache = {}


def _mm_dt():
    return {
        "float32": mybir.dt.float32,
        "float32r": mybir.dt.float32r,
        "bfloat16": mybir.dt.bfloat16,
        "float16": mybir.dt.float16,
    }[MM_DT_NAME]


def _np_dt(mdt):
    return mybir.dt.np(mdt)


def _patch_tile_teardown():
    """Drop the second all-engine barrier in TileContext's teardown: the
    sem-range clear runs on Pool after the first barrier; other engines
    need not wait for it (the runtime joins all engine streams at NEFF
    end anyway)."""
    from concourse.vector_clock import ScopedClock

    def _drain_and_barrier(self, tick_clock, wait_clock):
        drain_inst = self.nc.sync.drain()
        wait_clock.add_sem_waits(
            drain_inst.ins, ScopedClock({None: tick_clock.global_clock})
        )
        self.nc.all_engine_barrier()
        popped = self.nc._tile_sem_poison_stack.pop()
        assert popped is self._sem_poison
        self.nc.clear_and_free_semaphores(list(self.sems.allocated().values()))

    tile.TileContext._drain_and_barrier = _drain_and_barrier


_patch_tile_teardown()


def _build(mm_dt):
    # Skip the barrier Bass.__init__ emits after its const-pool memsets —
    # this kernel never reads the const pool from another engine.
    orig_barrier = bacc.Bacc.all_engine_barrier
    bacc.Bacc.all_engine_barrier = lambda self, **kw: None
    try:
        nc = bacc.Bacc(
            "TRN2", target_bir_lowering=False, debug=False, num_devices=N_CORES
        )
    finally:
        bacc.Bacc.all_engine_barrier = orig_barrier
    # batch-0 patch and the weights travel in ONE DMA (wt appended on the
    # free dim) so the weight packets don't compete with xt0's on the
    # SDMA engines
    xtw_d = nc.dram_tensor(
        "xtw", [PPAD, TROWS * RCOLS + 3 * OC], mm_dt, kind="ExternalInput"
    ).ap()
    xt1_d = nc.dram_tensor(
        "xt1", [PPAD, TROWS * RCOLS], mm_dt, kind="ExternalInput"
    ).ap()
    out_d = nc.dram_tensor(
        "out", [BPC, OC, RROWS, RCOLS], mybir.dt.float32, kind="ExternalOutput"
    ).ap()

    with tile.TileContext(nc) as tc, ExitStack() as ctx:
        xt_pool = ctx.enter_context(tc.tile_pool(name="xt", bufs=1))
        ps_pool = ctx.enter_context(tc.tile_pool(name="ps", bufs=6, space="PSUM"))
        ob_pool = ctx.enter_context(tc.tile_pool(name="ob", bufs=6))

        # Warm-up tile memset on ACT (scalar) — earliest free engine; the
        # PE warm-up matmuls flip the HAM clock gate (1.2 -> 2.4 GHz)
        # during the DMA wait so the real matmuls all run warm.
        # raw (non-pool) SBUF buffer: contents are garbage, which is fine
        # for warm-up matmuls, and needs no producing write to schedule
        NWARM = 374
        warm = nc.alloc_sbuf_tensor("warmbuf", [PPAD, NWARM], mm_dt).ap()

        # Inputs: both on gpsimd (SWDGE) — its queue starts emitting
        # earliest after the preamble, and same-queue FIFO drains batch 0
        # (+weights) before batch 1, so batch-0 compute starts at half the
        # transfer time. 128-partition transfers stripe across all 16
        # SDMA engines.
        xtw = xt_pool.tile([PPAD, TROWS * RCOLS + 3 * OC], mm_dt, tag="xtw")
        nc.sync.dma_start(out=xtw[:, :], in_=xtw_d[:, :])
        xt1 = xt_pool.tile([PPAD, TROWS * RCOLS], mm_dt, tag="xt1")
        nc.sync.dma_start(out=xt1[:, :], in_=xt1_d[:, :])
        wt = xtw[:, TROWS * RCOLS : TROWS * RCOLS + 3 * OC]
        xts = [xtw, xt1]

        wps = ps_pool.tile([OC, NWARM], mybir.dt.float32, tag="warmps", bufs=1)
        for _ in range(N_WARMUP):
            nc.tensor.matmul(
                wps[:, :], warm[:, 0:OC], warm[:, :], start=True, stop=True
            )

        for b in range(BPC):
            xt = xts[b]
            for ch, (r0, nr) in enumerate(CHUNKS):
                nf = nr * RCOLS
                ps = ps_pool.tile([OC, NFREE], mybir.dt.float32)
                for u in range(3):
                    # chunk covers p = 33+r0 .. +nr-1; tile row j = p-u-31
                    j0 = 2 + r0 - u
                    kk = KPART if u == 0 else KPART - 1
                    nc.tensor.matmul(
                        ps[:, 0:nf],
                        wt[0:kk, u * OC : (u + 1) * OC],
                        xt[0:kk, j0 * RCOLS : j0 * RCOLS + nf],
                        start=(u == 0),
                        stop=(u == 2),
                    )
                ob = ob_pool.tile([OC, NFREE], mybir.dt.float32)
                # alternate psum->sbuf copies between DVE and ACT so the
                # copy chain doesn't serialize behind the matmuls
                if ch % 2 == 0:
                    nc.vector.tensor_copy(ob[:, 0:nf], ps[:, 0:nf])
                else:
                    nc.scalar.copy(ob[:, 0:nf], ps[:, 0:nf])
                (nc.sync if ch % 2 == 0 else nc.scalar).dma_start(
                    out=out_d[b, :, r0 : r0 + nr, :],
                    in_=ob[:, 0:nf].rearrange("p (r c) -> p r c", c=RCOLS),
                )
    nc.compile()
    return nc


def _get_nc():
    key = MM_DT_NAME
    if key not in _cache:
        _cache[key] = _build(_mm_dt())
    return _cache[key]


LAST_RESULTS = None


def kernel(x, weight, bias):
    global LAST_RESULTS
    x = np.asarray(x, dtype=np.float32)
    weight = np.asarray(weight, dtype=np.float32)
    bias = np.asarray(bias, dtype=np.float32)
    np_dt = _np_dt(_mm_dt())

    # --- host prep: shard + im2col-lite (3 column-shifted replicas) ---
    xpad = np.zeros((B, IC, H + 2, W + 2), np.float32)
    xpad[:, :, :H, :W] = x
    XT = np.zeros((B, PPAD, TROWS, RCOLS), np.float32)
    for v in range(3):
        XT[:, v * 32 : (v + 1) * 32, :, :] = xpad[
            :, :, 31 : 31 + TROWS, 32 - v : 32 - v + RCOLS
        ]
    XT[:, 96] = 1.0
    XT = np.ascontiguousarray(XT.reshape(B, PPAD, TROWS * RCOLS)).astype(np_dt)

    WT = np.zeros((PPAD, 3 * OC), np.float32)
    # WT[v*32+i, u*64+oc] = weight[oc,i,u,v]
    WT[:96, :] = weight.transpose(3, 1, 2, 0).reshape(96, 3 * OC)
    WT[96, 0:OC] = bias
    WT = WT.astype(np_dt)

    nc = _get_nc()
    in_maps = [
        {
            "xtw": np.ascontiguousarray(
                np.concatenate([XT[c * BPC], WT], axis=1)
            ),
            "xt1": XT[c * BPC + 1],
        }
        for c in range(N_CORES)
    ]
    res = run_bass_kernel_spmd(nc, in_maps, list(range(N_CORES)))
    LAST_RESULTS = res

    dev = np.stack([r["out"] for r in res.results])  # [8, BPC, OC, 33, 34]
    dev = dev.reshape(B, OC, RROWS, RCOLS)

    # --- host assembly: bias everywhere, conv region flipped in ---
    full = np.empty((B, OC, H, W), np.float32)
    full[:] = bias[None, :, None, None]
    full[:, :, 31:64, 0:34] = dev[:, :, ::-1, :]
    return full
